# revision 36
# baseline (speedup 1.0000x reference)
"""Trainium2 Bass kernel v3 for DeepSet MLP (embedding-lookup-sum + MLP).

Math: u[b] = sum_j W_phi[x[b,j]] + N*b_phi; y = relu(relu(u@W1+b1)@W2+b2)@W3+b3.

Scheme (per core, 512 rows, data-parallel over 8 cores):
  - Class split c = 32*hi + lo. Host sends pre-transposed split index
    tensors xlo/xhi (bf16) in quad layout [j%128, (blk, i4, q, r4)].
  - One-hots hq[p,(i,q,lo,r)] / gq[p,(i,q,h,r)] built per block:
    most candidates on DVE (tensor_scalar is_equal runs in the 4x DVE
    perf mode: ~0.26 ns/elem marginal + ~100 ns/op overhead, so blocks
    are large: [48,48,32] quads -> 144 ops instead of 192), and the
    last K_OFFH hi-candidates on the ACT engine via the exact integer
    identity [v==h] = relu(1 - (v-h)^2)  (Square then Relu, table-set
    "small" has both).
  - Count matmuls: per 4-row quad, NI=4 j-chunk matmuls accumulate in
    PSUM: out[(lo,r), (q8,h,r')] holds complete counts on the r==r'
    diagonal. Whole-bank ACT evacuation to bf16 (counts are small ints).
  - Projection u^T = counts @ W_phi via diagonal-absorbing stationaries
    (wselz), sliced PER BLOCK so it pipelines with later blocks' counts.
    Output column order is block-major: col = blkbase + r*QB + q.
  - MLP per block (moving operand = contiguous 4*QB column slice), bf16
    with mean-centered activations; relu+bias on ACT, centering adds on
    DVE (4x). Host folds the N*b_phi path into per-layer f32 biases.
"""

import os
import numpy as np
from contextlib import ExitStack

import concourse.bass as bass
import concourse.bacc as bacc
import concourse.tile as tile
import concourse.mybir as mybir
from concourse.bass_utils import run_bass_kernel_spmd

B, N, C, PHI = 4096, 512, 512, 128
H1, H2 = 512, 256
NCORES = 8
BS = B // NCORES          # 512 rows per core
NI = 4                    # j chunks of 128
LO, HI = 32, 16

QS = [int(v) for v in os.environ.get("K_QS", "48,48,32").split(",")]
assert sum(QS) == 128 and all(q % 8 == 0 for q in QS)
NBLK = len(QS)
QSTART = [sum(QS[:i]) for i in range(NBLK)]
QMAX = max(QS)
_offs = os.environ.get("K_OFFS", "")
if _offs:
    OFFS = [int(v) for v in _offs.split(",")]
else:
    OFFS = [int(os.environ.get("K_OFFH", "2"))] * NBLK  # ACT-offloaded hi cands

F32 = mybir.dt.float32
BF16 = mybir.dt.bfloat16
ALU = mybir.AluOpType
AF = mybir.ActivationFunctionType


def build_program():
    nc = bacc.Bacc("TRN2", target_bir_lowering=False, debug=False,
                   num_devices=NCORES)

    xin_d = nc.dram_tensor("xin", [128, 2 * NI * 128 * 4], BF16,
                           kind="ExternalInput")
    wselz = nc.dram_tensor("wselz", [128, 4 * HI * PHI], BF16,
                           kind="ExternalInput")
    w1 = nc.dram_tensor("w1", [PHI, H1], BF16, kind="ExternalInput")
    w2 = nc.dram_tensor("w2", [128, 4 * 2 * 128], BF16, kind="ExternalInput")
    # packed small consts [128, 32] f32:
    # cols 0-3 b1c | 4-7 c1n | 8-9 b2c | 10-11 c2n | 12 off0 | 13 b3p(bcast)
    # | 14 one1 | 15 unused | 16-31 hbias; w3 bf16 packed in col 15 bits n/a
    cpack = nc.dram_tensor("cpack", [128, 40], F32, kind="ExternalInput")
    w3 = nc.dram_tensor("w3", [128, 2], BF16, kind="ExternalInput")
    out = nc.dram_tensor("out", [1, BS], F32, kind="ExternalOutput")

    with tile.TileContext(nc) as tc:
        with ExitStack() as ctx:
            _emit(ctx, tc, nc, xin_d, wselz, w1, w2, w3, cpack, out)
    nc.compile()
    return nc


def _emit(ctx, tc, nc, xin_d, wselz, w1, w2, w3, cpack, out):
    consts = ctx.enter_context(tc.tile_pool(name="consts", bufs=1))
    ohp = ctx.enter_context(tc.tile_pool(name="ohp", bufs=2))
    mlp = ctx.enter_context(tc.tile_pool(name="mlp", bufs=1))
    xin = ctx.enter_context(tc.tile_pool(name="xin", bufs=1))
    ps_cnt = ctx.enter_context(tc.tile_pool(name="ps_cnt", bufs=2,
                                            space="PSUM"))
    ps_u = ctx.enter_context(tc.tile_pool(name="ps_u", bufs=1, space="PSUM"))
    ps_mlp = ctx.enter_context(tc.tile_pool(name="ps_mlp", bufs=2,
                                            space="PSUM"))
    ps_w2 = ctx.enter_context(tc.tile_pool(name="ps_w2", bufs=1,
                                           space="PSUM"))
    ps_y = ctx.enter_context(tc.tile_pool(name="ps_y", bufs=1, space="PSUM"))
    hup = ctx.enter_context(tc.tile_pool(name="hup", bufs=2))

    # Index tensors first: the one-hot pipeline depends only on these.
    xbs = [xin.tile([128, 2 * NI * QS[b] * 4], BF16, name=f"xb{b}")
           for b in range(NBLK)]
    for b in range(NBLK):
        o0 = 2 * NI * QSTART[b] * 4
        o1 = 2 * NI * (QSTART[b] + QS[b]) * 4
        if b == 0:
            mid = NI * QS[b] * 4
            nc.sync.dma_start(xbs[b][:, :mid], xin_d.ap()[:, o0:o0 + mid])
            nc.sync.dma_start(xbs[b][:, mid:], xin_d.ap()[:, o0 + mid:o1])
        else:
            nc.sync.dma_start(xbs[b][:], xin_d.ap()[:, o0:o1])
    xls = [xbs[b][:, :NI * QS[b] * 4] for b in range(NBLK)]
    xhs = [xbs[b][:, NI * QS[b] * 4:] for b in range(NBLK)]
    cp = consts.tile([128, 40], F32)
    nc.sync.dma_start(cp[:], cpack.ap())
    w3sb = consts.tile([128, 2], BF16)
    nc.sync.dma_start(w3sb[:], w3.ap())
    wselz_sb = consts.tile([128, 4 * HI * PHI], BF16)
    nc.sync.dma_start(wselz_sb[:], wselz.ap())
    w1sb = consts.tile([128, H1], BF16)
    nc.sync.dma_start(w1sb[:], w1.ap())
    w2sb = consts.tile([128, 4 * 2 * 128], BF16)
    nc.sync.dma_start(w2sb[:], w2.ap())
    b1csb = cp[:, 0:4]
    c1nsb = cp[:, 4:8]
    b2csb = cp[:, 8:10]
    c2nsb = cp[:, 10:12]
    off0sb = cp[:, 12:13]
    b3psb = cp[:, 13:14]
    one1_sb = cp[:, 14:15]
    hbias_sb = cp[:, 16:32]
    b1psb = cp[:, 32:36]
    b2psb = cp[:, 36:38]

    # counts: [128=(lo,r), (h16, r'4, q128)] bf16, h/r'-major so the
    # projection's moving columns are contiguous slices per block.
    fvq = mlp.tile([128, 128 * HI * 4], BF16)
    fvh = fvq[:].rearrange("p (h r q) -> p h r q", h=HI, r=4)

    ysb = mlp.tile([1, BS], F32, name="ysb")
    pu = ps_u.tile([128, BS], F32)
    usb = mlp.tile([128, BS], BF16)
    h1sb = [mlp.tile([128, BS], BF16, name=f"h1sb{k}") for k in range(4)]
    h2sb = [mlp.tile([128, BS], BF16, name=f"h2sb{k}") for k in range(2)]
    w2v = w2sb[:].rearrange("p (kc m) -> p kc m", kc=4)

    # per-block state kept across phases
    st = [dict() for _ in range(NBLK)]

    def emit_oh(blk):
        QB = QS[blk]
        xl = xls[blk]
        xh = xhs[blk]
        hq = ohp.tile([128, NI * QMAX * LO * 4], BF16, tag="hq")
        gq = ohp.tile([128, NI * QMAX * HI * 4], BF16, tag="gq")
        sq = ohp.tile([128, NI * QMAX * 4], BF16, tag="sq")
        st[blk]["hq"], st[blk]["gq"] = hq, gq
        hqv = hq[:].rearrange("p (i q l r) -> p i q l r", i=NI, q=QMAX, l=LO)
        hqv = hqv[:, :, :QB, :, :]
        gqv = gq[:].rearrange("p (i q l r) -> p i q l r", i=NI, q=QMAX, l=HI)
        gqv = gqv[:, :, :QB, :, :]
        xlv = xl.rearrange("p (i q r) -> p i q r", i=NI, q=QB)
        xhv = xh.rearrange("p (i q r) -> p i q r", i=NI, q=QB)

        # ACT-offloaded hi candidates first (slow engine starts early):
        # [v==h] = relu(1 - (v-h)^2), exact for integers.
        for h in range(HI - OFFS[blk], HI):
            nc.scalar.activation(sq[:, :NI * QB * 4], xh, AF.Square,
                                 bias=hbias_sb[:, h:h + 1], scale=1.0)
            nc.scalar.activation(gqv[:, :, :, h, :],
                                 sq[:, :NI * QB * 4].rearrange(
                                     "p (i q r) -> p i q r", i=NI, q=QB),
                                 AF.Relu, bias=one1_sb[:, 0:1], scale=-1.0)
        for lo in range(LO):
            nc.vector.tensor_scalar(out=hqv[:, :, :, lo, :], in0=xlv,
                                    scalar1=lo, scalar2=None,
                                    op0=ALU.is_equal)
        for h in range(HI - OFFS[blk]):
            nc.vector.tensor_scalar(out=gqv[:, :, :, h, :], in0=xhv,
                                    scalar1=h, scalar2=None,
                                    op0=ALU.is_equal)

    def emit_counts(blk):
        QB = QS[blk]
        hqm = st[blk]["hq"][:].rearrange("p (i q c) -> p i q c", i=NI, q=QMAX)
        gqm = st[blk]["gq"][:].rearrange("p (i q c) -> p i q c", i=NI, q=QMAX)
        pts = []
        for t in range(QB // 8):
            pt = ps_cnt.tile([128, 512], F32, tag="cnt")
            pts.append(pt)
            for q8 in range(8):
                qb = t * 8 + q8
                for i in range(NI):
                    nc.tensor.matmul(
                        pt[:, q8 * 64:(q8 + 1) * 64],
                        hqm[:, i, qb, :],
                        gqm[:, i, qb, :],
                        start=(i == 0), stop=(i == NI - 1))
            # evacuate each 8-quad group as soon as its psum closes:
            # psum (q8, h, r') -> fvq (h, r', q)
            bq0 = QSTART[blk] + t * 8
            dst = fvh[:, :, :, bq0:bq0 + 8]
            srcap = pt[:].rearrange("p (q h r) -> p h r q", q=8, h=HI)
            nc.scalar.copy(dst, srcap)

    def emit_proj(blk):
        QB = QS[blk]
        q0 = QSTART[blk]
        cbase = 4 * q0
        for r in range(4):
            cs = cbase + r * QB
            for h in range(HI):
                nc.tensor.matmul(
                    pu[:, cs:cs + QB],
                    wselz_sb[:, (r * HI + h) * PHI:(r * HI + h + 1) * PHI],
                    fvh[:, h, r, q0:q0 + QB],
                    start=(h == 0), stop=(h == HI - 1))

    def emit_cons_a(blk, nblk=1):
        cbase = 4 * QSTART[blk]
        CB = 4 * sum(QS[blk:blk + nblk])
        nc.vector.tensor_scalar(out=usb[:, cbase:cbase + CB],
                                in0=pu[:, cbase:cbase + CB],
                                scalar1=off0sb[:, 0:1],
                                scalar2=None, op0=ALU.add)
        for kc in range(4):
            ph = ps_mlp.tile([128, CB], F32, tag="ph", name="ph_a")
            nc.tensor.matmul(ph[:], w1sb[:, kc * 128:(kc + 1) * 128],
                             usb[:, cbase:cbase + CB],
                             start=True, stop=True)
            # h1sb = relu(ph + b1p) - c1  ==  max(ph + (b1p-c1), -c1)
            nc.vector.tensor_scalar(out=h1sb[kc][:, cbase:cbase + CB],
                                    in0=ph[:],
                                    scalar1=b1csb[:, kc:kc + 1],
                                    scalar2=c1nsb[:, kc:kc + 1],
                                    op0=ALU.add, op1=ALU.max)

    def emit_cons_b(blk, nblk=1):
        cbase = 4 * QSTART[blk]
        CB = 4 * sum(QS[blk:blk + nblk])
        ph2 = [ps_w2.tile([128, CB], F32, tag=f"ph2_{m}", name=f"ph2_{m}")
               for m in range(2)]
        for kc in range(4):
            for mc in range(2):
                nc.tensor.matmul(ph2[mc][:],
                                 w2v[:, kc, mc * 128:(mc + 1) * 128],
                                 h1sb[kc][:, cbase:cbase + CB],
                                 start=(kc == 0), stop=(kc == 3))
        for mc in range(2):
            nc.vector.tensor_scalar(out=h2sb[mc][:, cbase:cbase + CB],
                                    in0=ph2[mc][:],
                                    scalar1=b2csb[:, mc:mc + 1],
                                    scalar2=c2nsb[:, mc:mc + 1],
                                    op0=ALU.add, op1=ALU.max)
        py = ps_y.tile([1, CB], F32, tag="py", name="py")
        for mc in range(2):
            nc.tensor.matmul(py[:], w3sb[:, mc:mc + 1],
                             h2sb[mc][:, cbase:cbase + CB],
                             start=(mc == 0), stop=(mc == 1))
        nc.vector.tensor_scalar(out=ysb[0:1, cbase:cbase + CB],
                                in0=py[:], scalar1=b3psb[0:1, 0:1],
                                scalar2=None, op0=ALU.add)

    def emit_cons(blk, nblk=1):
        emit_cons_a(blk, nblk)
        emit_cons_b(blk, nblk)

    def emit_cons_act(blk, nblk=1):
        # ACT does usb and the h1 relu/center (2 ops via f32 scratch), so
        # the W1/W2 matmuls fill the PE idle window between proj1 and
        # counts of the last block instead of contending with its proj.
        cbase = 4 * QSTART[blk]
        CB = 4 * sum(QS[blk:blk + nblk])
        nc.scalar.activation(usb[:, cbase:cbase + CB],
                             pu[:, cbase:cbase + CB],
                             AF.Identity, bias=off0sb[:, 0:1], scale=1.0)
        ph2 = [ps_w2.tile([128, CB], F32, tag=f"ph2_{m}", name=f"ph2_{m}")
               for m in range(2)]
        for kc in range(4):
            ph = ps_mlp.tile([128, CB], F32, tag="ph", name="ph_a")
            nc.tensor.matmul(ph[:], w1sb[:, kc * 128:(kc + 1) * 128],
                             usb[:, cbase:cbase + CB],
                             start=True, stop=True)
            hu = hup.tile([128, 4 * QMAX * 2], F32, tag="hu")
            nc.scalar.activation(hu[:, :CB], ph[:], AF.Relu,
                                 bias=b1psb[:, kc:kc + 1], scale=1.0)
            nc.scalar.activation(h1sb[kc][:, cbase:cbase + CB], hu[:, :CB],
                                 AF.Identity, bias=c1nsb[:, kc:kc + 1],
                                 scale=1.0)
            for mc in range(2):
                nc.tensor.matmul(ph2[mc][:],
                                 w2v[:, kc, mc * 128:(mc + 1) * 128],
                                 h1sb[kc][:, cbase:cbase + CB],
                                 start=(kc == 0), stop=(kc == 3))
        for mc in range(2):
            nc.vector.tensor_scalar(out=h2sb[mc][:, cbase:cbase + CB],
                                    in0=ph2[mc][:],
                                    scalar1=b2csb[:, mc:mc + 1],
                                    scalar2=c2nsb[:, mc:mc + 1],
                                    op0=ALU.add, op1=ALU.max)
        py = ps_y.tile([1, CB], F32, tag="py", name="py")
        for mc in range(2):
            nc.tensor.matmul(py[:], w3sb[:, mc:mc + 1],
                             h2sb[mc][:, cbase:cbase + CB],
                             start=(mc == 0), stop=(mc == 1))
        nc.vector.tensor_scalar(out=ysb[0:1, cbase:cbase + CB],
                                in0=py[:], scalar1=b3psb[0:1, 0:1],
                                scalar2=None, op0=ALU.add)

    def emit_out():
        # ysb columns are block-major (blk | r, q); host un-permutes.
        nc.scalar.dma_start(out.ap(), ysb[:])

    # Software-pipelined emission: per-engine streams are in-order, so
    # consumer ops (DVE TS / ACT evac) must come after later blocks'
    # one-hot ops or they head-of-line-block them.
    if NBLK == 3:
        emit_oh(0)
        emit_oh(1)
        emit_counts(0)
        emit_oh(2)
        emit_counts(1)
        emit_proj(0)
        emit_proj(1)
        emit_counts(2)
        emit_cons(0, nblk=2)
        emit_proj(2)
        emit_cons(2)
        emit_out()
    elif NBLK == 4:
        emit_oh(0)
        emit_oh(1)
        emit_counts(0)
        emit_counts(1)
        emit_proj(0)
        emit_oh(2)
        emit_proj(1)
        emit_counts(2)
        emit_oh(3)
        emit_proj(2)
        emit_cons(0, nblk=3)
        emit_counts(3)
        emit_proj(3)
        emit_cons(3)
        emit_out()
    else:
        raise ValueError(NBLK)




_CACHED_NC = None


def _get_nc():
    global _CACHED_NC
    if _CACHED_NC is None:
        _CACHED_NC = build_program()
    return _CACHED_NC


def _prep_in_maps(x, W_phi, b_phi, W1, b1, W2, b2, W3, b3):
    import ml_dtypes
    bf = ml_dtypes.bfloat16

    x = np.asarray(x, dtype=np.int64)
    Wd = np.asarray(W_phi, np.float64)
    bphid = np.asarray(b_phi, np.float64)
    W1d = np.asarray(W1, np.float64)
    W2d = np.asarray(W2, np.float64)
    W3d = np.asarray(W3, np.float64)
    b1d, b2d, b3d = (np.asarray(v, np.float64) for v in (b1, b2, b3))

    Wb = Wd.astype(np.float32).astype(bf).astype(np.float64)
    ubar = (N / C) * Wb.sum(0) + N * bphid
    off0v = (N * bphid - ubar).astype(np.float32)
    b1pv = (b1d + ubar @ W1d).astype(np.float32)
    c1v = np.maximum(b1pv, 0.0)
    b2pv = (b2d + c1v.astype(np.float64) @ W2d).astype(np.float32)
    c2v = np.maximum(b2pv, 0.0)
    b3pv = (b3d + c2v.astype(np.float64) @ W3d).astype(np.float32)

    # wselz[(lo,r) partition, (rv, h, d)] = Wb[32h+lo, d] iff r == rv
    wsel = Wb.astype(np.float32).reshape(HI, LO, PHI)     # [h, lo, d]
    wz = np.zeros((LO, 4, 4, HI, PHI), np.float32)        # [lo, r, rv, h, d]
    for r in range(4):
        wz[:, r, r, :, :] = wsel.transpose(1, 0, 2)
    wselzv = np.ascontiguousarray(
        wz.reshape(128, 4 * HI * PHI).astype(bf))

    w1v = np.ascontiguousarray(W1d.astype(np.float32).astype(bf))
    w2vv = np.ascontiguousarray(
        W2d.astype(np.float32).reshape(4, 128, 2, 128).transpose(1, 0, 2, 3)
        .reshape(128, 1024).astype(bf))
    w3v = np.ascontiguousarray(
        W3d.astype(np.float32).reshape(2, 128).T.astype(bf))

    cpk = np.zeros((128, 40), np.float32)
    cpk[:, 0:4] = np.minimum(b1pv, 0.0).reshape(4, 128).T
    cpk[:, 4:8] = (-c1v).reshape(4, 128).T
    cpk[:, 8:10] = np.minimum(b2pv, 0.0).reshape(2, 128).T
    cpk[:, 10:12] = (-c2v).reshape(2, 128).T
    cpk[:, 12] = off0v
    cpk[:, 13] = float(np.asarray(b3pv).reshape(-1)[0])
    cpk[:, 14] = 1.0
    cpk[:, 16:32] = -np.arange(HI, dtype=np.float32)
    cpk[:, 32:36] = b1pv.reshape(4, 128).T
    cpk[:, 36:38] = b2pv.reshape(2, 128).T
    shared = {
        "wselz": wselzv,
        "w1": w1v,
        "w2": w2vv,
        "w3": w3v,
        "cpack": np.ascontiguousarray(cpk),
    }

    lo_all = (x & 31).astype(np.float32)
    hi_all = (x >> 5).astype(np.float32)

    def quadpack(v):  # [512b, 512j] -> [128p, concat_blk(i, q_b, r)]
        a = v.T.reshape(NI, 128, 128, 4)             # [i, p, g, r]
        a = a.transpose(1, 0, 2, 3)                  # [p, i, g, r]
        parts = [np.ascontiguousarray(a[:, :, s:s + n, :]).reshape(128, -1)
                 for s, n in zip(QSTART, QS)]
        return np.ascontiguousarray(
            np.concatenate(parts, axis=1).astype(bf))

    maps = []
    for c in range(NCORES):
        sl = slice(c * BS, (c + 1) * BS)
        m = dict(shared)
        ql, qh = quadpack(lo_all[sl]), quadpack(hi_all[sl])
        parts = []
        for b in range(NBLK):
            o0, o1 = NI * QSTART[b] * 4, NI * (QSTART[b] + QS[b]) * 4
            parts.append(ql[:, o0:o1])
            parts.append(qh[:, o0:o1])
        m["xin"] = np.ascontiguousarray(np.concatenate(parts, axis=1))
        maps.append(m)
    return maps


def _unpermute(y_core):
    """Device col (blk | r, q-local) -> row b = (QSTART[blk]+q)*4 + r."""
    outp = np.empty(BS, np.float32)
    for blk in range(NBLK):
        QB = QS[blk]
        seg = y_core[4 * QSTART[blk]:4 * (QSTART[blk] + QB)].reshape(4, QB)
        for r in range(4):
            outp[(QSTART[blk] + np.arange(QB)) * 4 + r] = seg[r]
    return outp


def run(trace=False, tmpdir=None, **inputs):
    nc = _get_nc()
    in_maps = _prep_in_maps(**inputs)
    res = run_bass_kernel_spmd(nc, in_maps, core_ids=list(range(NCORES)),
                               trace=trace, tmpdir=tmpdir)
    y = np.concatenate([
        _unpermute(np.asarray(res.results[c]["out"]).reshape(BS))
        for c in range(NCORES)])
    return y.reshape(B, 1).astype(np.float32), res


def kernel(**inputs):
    y, _ = run(trace=False, **inputs)
    return y


# revision 37
# speedup vs baseline: 1.0181x; 1.0181x over previous
"""Trainium2 Bass kernel v3 for DeepSet MLP (embedding-lookup-sum + MLP).

Math: u[b] = sum_j W_phi[x[b,j]] + N*b_phi; y = relu(relu(u@W1+b1)@W2+b2)@W3+b3.

Scheme (per core, 512 rows, data-parallel over 8 cores):
  - Class split c = 32*hi + lo. Host sends pre-transposed split index
    tensors xlo/xhi (bf16) in quad layout [j%128, (blk, i4, q, r4)].
  - One-hots hq[p,(i,q,lo,r)] / gq[p,(i,q,h,r)] built per block:
    most candidates on DVE (tensor_scalar is_equal runs in the 4x DVE
    perf mode: ~0.26 ns/elem marginal + ~100 ns/op overhead, so blocks
    are large: [48,48,32] quads -> 144 ops instead of 192), and the
    last K_OFFH hi-candidates on the ACT engine via the exact integer
    identity [v==h] = relu(1 - (v-h)^2)  (Square then Relu, table-set
    "small" has both).
  - Count matmuls: per 4-row quad, NI=4 j-chunk matmuls accumulate in
    PSUM: out[(lo,r), (q8,h,r')] holds complete counts on the r==r'
    diagonal. Whole-bank ACT evacuation to bf16 (counts are small ints).
  - Projection u^T = counts @ W_phi via diagonal-absorbing stationaries
    (wselz), sliced PER BLOCK so it pipelines with later blocks' counts.
    Output column order is block-major: col = blkbase + r*QB + q.
  - MLP per block (moving operand = contiguous 4*QB column slice), bf16
    with mean-centered activations; relu+bias on ACT, centering adds on
    DVE (4x). Host folds the N*b_phi path into per-layer f32 biases.
"""

import os
import numpy as np
from contextlib import ExitStack

import concourse.bass as bass
import concourse.bacc as bacc
import concourse.tile as tile
import concourse.mybir as mybir
from concourse.bass_utils import run_bass_kernel_spmd

B, N, C, PHI = 4096, 512, 512, 128
H1, H2 = 512, 256
NCORES = 8
BS = B // NCORES          # 512 rows per core
NI = 4                    # j chunks of 128
LO, HI = 32, 16

QS = [int(v) for v in os.environ.get("K_QS", "48,48,32").split(",")]
assert sum(QS) == 128 and all(q % 8 == 0 for q in QS)
NBLK = len(QS)
QSTART = [sum(QS[:i]) for i in range(NBLK)]
QMAX = max(QS)
_offs = os.environ.get("K_OFFS", "")
if _offs:
    OFFS = [int(v) for v in _offs.split(",")]
else:
    OFFS = [int(os.environ.get("K_OFFH", "2"))] * NBLK  # ACT-offloaded hi cands

F32 = mybir.dt.float32
BF16 = mybir.dt.bfloat16
ALU = mybir.AluOpType
AF = mybir.ActivationFunctionType


def build_program():
    nc = bacc.Bacc("TRN2", target_bir_lowering=False, debug=False,
                   num_devices=NCORES)

    xin_d = nc.dram_tensor("xin", [128, 2 * NI * 128 * 4], BF16,
                           kind="ExternalInput")
    wselz = nc.dram_tensor("wselz", [128, 4 * HI * PHI], BF16,
                           kind="ExternalInput")
    w1 = nc.dram_tensor("w1", [PHI, H1], BF16, kind="ExternalInput")
    w2 = nc.dram_tensor("w2", [128, 4 * 2 * 128], BF16, kind="ExternalInput")
    # packed small consts [128, 32] f32:
    # cols 0-3 b1c | 4-7 c1n | 8-9 b2c | 10-11 c2n | 12 off0 | 13 b3p(bcast)
    # | 14 one1 | 15 unused | 16-31 hbias; w3 bf16 packed in col 15 bits n/a
    cpack = nc.dram_tensor("cpack", [128, 40], F32, kind="ExternalInput")
    w3 = nc.dram_tensor("w3", [128, 2], BF16, kind="ExternalInput")
    out = nc.dram_tensor("out", [1, BS], F32, kind="ExternalOutput")

    with tile.TileContext(nc) as tc:
        with ExitStack() as ctx:
            _emit(ctx, tc, nc, xin_d, wselz, w1, w2, w3, cpack, out)
    nc.compile()
    return nc


def _emit(ctx, tc, nc, xin_d, wselz, w1, w2, w3, cpack, out):
    consts = ctx.enter_context(tc.tile_pool(name="consts", bufs=1))
    ohp = ctx.enter_context(tc.tile_pool(name="ohp", bufs=2))
    mlp = ctx.enter_context(tc.tile_pool(name="mlp", bufs=1))
    xin = ctx.enter_context(tc.tile_pool(name="xin", bufs=1))
    ps_cnt = ctx.enter_context(tc.tile_pool(name="ps_cnt", bufs=2,
                                            space="PSUM"))
    ps_u = ctx.enter_context(tc.tile_pool(name="ps_u", bufs=1, space="PSUM"))
    ps_mlp = ctx.enter_context(tc.tile_pool(name="ps_mlp", bufs=2,
                                            space="PSUM"))
    ps_w2 = ctx.enter_context(tc.tile_pool(name="ps_w2", bufs=1,
                                           space="PSUM"))
    ps_y = ctx.enter_context(tc.tile_pool(name="ps_y", bufs=1, space="PSUM"))
    hup = ctx.enter_context(tc.tile_pool(name="hup", bufs=2))

    # Index tensors first: the one-hot pipeline depends only on these.
    xbs = [xin.tile([128, 2 * NI * QS[b] * 4], BF16, name=f"xb{b}")
           for b in range(NBLK)]
    for b in range(NBLK):
        o0 = 2 * NI * QSTART[b] * 4
        o1 = 2 * NI * (QSTART[b] + QS[b]) * 4
        if b == 0:
            mid = NI * QS[b] * 4
            nc.sync.dma_start(xbs[b][:, :mid], xin_d.ap()[:, o0:o0 + mid])
            nc.sync.dma_start(xbs[b][:, mid:], xin_d.ap()[:, o0 + mid:o1])
        else:
            nc.sync.dma_start(xbs[b][:], xin_d.ap()[:, o0:o1])
    xls = [xbs[b][:, :NI * QS[b] * 4] for b in range(NBLK)]
    xhs = [xbs[b][:, NI * QS[b] * 4:] for b in range(NBLK)]
    cp = consts.tile([128, 40], F32)
    nc.sync.dma_start(cp[:], cpack.ap())
    w3sb = consts.tile([128, 2], BF16)
    nc.sync.dma_start(w3sb[:], w3.ap())
    wselz_sb = consts.tile([128, 4 * HI * PHI], BF16)
    nc.sync.dma_start(wselz_sb[:], wselz.ap())
    w1sb = consts.tile([128, H1], BF16)
    nc.sync.dma_start(w1sb[:], w1.ap())
    w2sb = consts.tile([128, 4 * 2 * 128], BF16)
    nc.sync.dma_start(w2sb[:], w2.ap())
    b1csb = cp[:, 0:4]
    c1nsb = cp[:, 4:8]
    b2csb = cp[:, 8:10]
    c2nsb = cp[:, 10:12]
    off0sb = cp[:, 12:13]
    b3psb = cp[:, 13:14]
    one1_sb = cp[:, 14:15]
    hbias_sb = cp[:, 16:32]
    b1psb = cp[:, 32:36]
    b2psb = cp[:, 36:38]

    # counts: [128=(lo,r), (h16, r'4, q128)] bf16, h/r'-major so the
    # projection's moving columns are contiguous slices per block.
    fvq = mlp.tile([128, 128 * HI * 4], BF16)
    fvh = fvq[:].rearrange("p (h r q) -> p h r q", h=HI, r=4)

    ysb = mlp.tile([1, BS], F32, name="ysb")
    pu = ps_u.tile([128, BS], F32)
    usb = mlp.tile([128, BS], BF16)
    h1sb = [mlp.tile([128, BS], BF16, name=f"h1sb{k}") for k in range(4)]
    h2sb = [mlp.tile([128, BS], BF16, name=f"h2sb{k}") for k in range(2)]
    w2v = w2sb[:].rearrange("p (kc m) -> p kc m", kc=4)

    # per-block state kept across phases
    st = [dict() for _ in range(NBLK)]

    def emit_oh(blk):
        QB = QS[blk]
        xl = xls[blk]
        xh = xhs[blk]
        hq = ohp.tile([128, NI * QMAX * LO * 4], BF16, tag="hq")
        gq = ohp.tile([128, NI * QMAX * HI * 4], BF16, tag="gq")
        sq = ohp.tile([128, NI * QMAX * 4], BF16, tag="sq")
        st[blk]["hq"], st[blk]["gq"] = hq, gq
        hqv = hq[:].rearrange("p (i q l r) -> p i q l r", i=NI, q=QMAX, l=LO)
        hqv = hqv[:, :, :QB, :, :]
        gqv = gq[:].rearrange("p (i q l r) -> p i q l r", i=NI, q=QMAX, l=HI)
        gqv = gqv[:, :, :QB, :, :]
        xlv = xl.rearrange("p (i q r) -> p i q r", i=NI, q=QB)
        xhv = xh.rearrange("p (i q r) -> p i q r", i=NI, q=QB)

        # ACT-offloaded hi candidates first (slow engine starts early):
        # [v==h] = relu(1 - (v-h)^2), exact for integers.
        for h in range(HI - OFFS[blk], HI):
            nc.scalar.activation(sq[:, :NI * QB * 4], xh, AF.Square,
                                 bias=hbias_sb[:, h:h + 1], scale=1.0)
            nc.scalar.activation(gqv[:, :, :, h, :],
                                 sq[:, :NI * QB * 4].rearrange(
                                     "p (i q r) -> p i q r", i=NI, q=QB),
                                 AF.Relu, bias=one1_sb[:, 0:1], scale=-1.0)
        for lo in range(LO):
            nc.vector.tensor_scalar(out=hqv[:, :, :, lo, :], in0=xlv,
                                    scalar1=lo, scalar2=None,
                                    op0=ALU.is_equal)
        for h in range(HI - OFFS[blk]):
            nc.vector.tensor_scalar(out=gqv[:, :, :, h, :], in0=xhv,
                                    scalar1=h, scalar2=None,
                                    op0=ALU.is_equal)

    def emit_counts(blk):
        QB = QS[blk]
        hqm = st[blk]["hq"][:].rearrange("p (i q c) -> p i q c", i=NI, q=QMAX)
        gqm = st[blk]["gq"][:].rearrange("p (i q c) -> p i q c", i=NI, q=QMAX)
        pts = []
        for t in range(QB // 8):
            pt = ps_cnt.tile([128, 512], F32, tag="cnt")
            pts.append(pt)
            for q8 in range(8):
                qb = t * 8 + q8
                for i in range(NI):
                    nc.tensor.matmul(
                        pt[:, q8 * 64:(q8 + 1) * 64],
                        hqm[:, i, qb, :],
                        gqm[:, i, qb, :],
                        start=(i == 0), stop=(i == NI - 1))
            # evacuate each 8-quad group as soon as its psum closes:
            # psum (q8, h, r') -> fvq (h, r', q)
            bq0 = QSTART[blk] + t * 8
            dst = fvh[:, :, :, bq0:bq0 + 8]
            srcap = pt[:].rearrange("p (q h r) -> p h r q", q=8, h=HI)
            nc.scalar.copy(dst, srcap)

    def emit_proj(blk):
        QB = QS[blk]
        q0 = QSTART[blk]
        cbase = 4 * q0
        for r in range(4):
            cs = cbase + r * QB
            for h in range(HI):
                nc.tensor.matmul(
                    pu[:, cs:cs + QB],
                    wselz_sb[:, (r * HI + h) * PHI:(r * HI + h + 1) * PHI],
                    fvh[:, h, r, q0:q0 + QB],
                    start=(h == 0), stop=(h == HI - 1))

    def emit_cons_a(blk, nblk=1):
        cbase = 4 * QSTART[blk]
        CB = 4 * sum(QS[blk:blk + nblk])
        nc.vector.tensor_scalar(out=usb[:, cbase:cbase + CB],
                                in0=pu[:, cbase:cbase + CB],
                                scalar1=off0sb[:, 0:1],
                                scalar2=None, op0=ALU.add)
        for kc in range(4):
            ph = ps_mlp.tile([128, CB], F32, tag="ph", name="ph_a")
            nc.tensor.matmul(ph[:], w1sb[:, kc * 128:(kc + 1) * 128],
                             usb[:, cbase:cbase + CB],
                             start=True, stop=True)
            # h1sb = relu(ph + b1p) - c1  ==  max(ph + (b1p-c1), -c1)
            nc.vector.tensor_scalar(out=h1sb[kc][:, cbase:cbase + CB],
                                    in0=ph[:],
                                    scalar1=b1csb[:, kc:kc + 1],
                                    scalar2=c1nsb[:, kc:kc + 1],
                                    op0=ALU.add, op1=ALU.max)

    def emit_cons_b(blk, nblk=1):
        cbase = 4 * QSTART[blk]
        CB = 4 * sum(QS[blk:blk + nblk])
        ph2 = [ps_w2.tile([128, CB], F32, tag=f"ph2_{m}", name=f"ph2_{m}")
               for m in range(2)]
        for kc in range(4):
            for mc in range(2):
                nc.tensor.matmul(ph2[mc][:],
                                 w2v[:, kc, mc * 128:(mc + 1) * 128],
                                 h1sb[kc][:, cbase:cbase + CB],
                                 start=(kc == 0), stop=(kc == 3))
        for mc in range(2):
            nc.vector.tensor_scalar(out=h2sb[mc][:, cbase:cbase + CB],
                                    in0=ph2[mc][:],
                                    scalar1=b2csb[:, mc:mc + 1],
                                    scalar2=c2nsb[:, mc:mc + 1],
                                    op0=ALU.add, op1=ALU.max)
        py = ps_y.tile([1, CB], F32, tag="py", name="py")
        for mc in range(2):
            nc.tensor.matmul(py[:], w3sb[:, mc:mc + 1],
                             h2sb[mc][:, cbase:cbase + CB],
                             start=(mc == 0), stop=(mc == 1))
        nc.vector.tensor_scalar(out=ysb[0:1, cbase:cbase + CB],
                                in0=py[:], scalar1=b3psb[0:1, 0:1],
                                scalar2=None, op0=ALU.add)

    def emit_cons(blk, nblk=1):
        emit_cons_a(blk, nblk)
        emit_cons_b(blk, nblk)

    def emit_cons_act(blk, nblk=1):
        # ACT does usb and the h1 relu/center (2 ops via f32 scratch), so
        # the W1/W2 matmuls fill the PE idle window between proj1 and
        # counts of the last block instead of contending with its proj.
        cbase = 4 * QSTART[blk]
        CB = 4 * sum(QS[blk:blk + nblk])
        nc.scalar.activation(usb[:, cbase:cbase + CB],
                             pu[:, cbase:cbase + CB],
                             AF.Identity, bias=off0sb[:, 0:1], scale=1.0)
        ph2 = [ps_w2.tile([128, CB], F32, tag=f"ph2_{m}", name=f"ph2_{m}")
               for m in range(2)]
        for kc in range(4):
            ph = ps_mlp.tile([128, CB], F32, tag="ph", name="ph_a")
            nc.tensor.matmul(ph[:], w1sb[:, kc * 128:(kc + 1) * 128],
                             usb[:, cbase:cbase + CB],
                             start=True, stop=True)
            hu = hup.tile([128, 4 * QMAX * 2], F32, tag="hu")
            nc.scalar.activation(hu[:, :CB], ph[:], AF.Relu,
                                 bias=b1psb[:, kc:kc + 1], scale=1.0)
            nc.scalar.activation(h1sb[kc][:, cbase:cbase + CB], hu[:, :CB],
                                 AF.Identity, bias=c1nsb[:, kc:kc + 1],
                                 scale=1.0)
            for mc in range(2):
                nc.tensor.matmul(ph2[mc][:],
                                 w2v[:, kc, mc * 128:(mc + 1) * 128],
                                 h1sb[kc][:, cbase:cbase + CB],
                                 start=(kc == 0), stop=(kc == 3))
        for mc in range(2):
            nc.vector.tensor_scalar(out=h2sb[mc][:, cbase:cbase + CB],
                                    in0=ph2[mc][:],
                                    scalar1=b2csb[:, mc:mc + 1],
                                    scalar2=c2nsb[:, mc:mc + 1],
                                    op0=ALU.add, op1=ALU.max)
        py = ps_y.tile([1, CB], F32, tag="py", name="py")
        for mc in range(2):
            nc.tensor.matmul(py[:], w3sb[:, mc:mc + 1],
                             h2sb[mc][:, cbase:cbase + CB],
                             start=(mc == 0), stop=(mc == 1))
        nc.vector.tensor_scalar(out=ysb[0:1, cbase:cbase + CB],
                                in0=py[:], scalar1=b3psb[0:1, 0:1],
                                scalar2=None, op0=ALU.add)

    def emit_out():
        # ysb columns are block-major (blk | r, q); host un-permutes.
        nc.scalar.dma_start(out.ap(), ysb[:])

    # Software-pipelined emission: per-engine streams are in-order, so
    # consumer ops (DVE TS / ACT evac) must come after later blocks'
    # one-hot ops or they head-of-line-block them.
    if NBLK == 3:
        emit_oh(0)
        emit_oh(1)
        emit_counts(0)
        emit_counts(1)
        emit_proj(0)
        emit_oh(2)
        emit_proj(1)
        emit_counts(2)
        emit_cons(0, nblk=2)
        emit_proj(2)
        emit_cons(2)
        emit_out()
    elif NBLK == 4:
        emit_oh(0)
        emit_oh(1)
        emit_counts(0)
        emit_counts(1)
        emit_proj(0)
        emit_oh(2)
        emit_proj(1)
        emit_counts(2)
        emit_oh(3)
        emit_proj(2)
        emit_cons(0, nblk=3)
        emit_counts(3)
        emit_proj(3)
        emit_cons(3)
        emit_out()
    else:
        raise ValueError(NBLK)




_CACHED_NC = None


def _get_nc():
    global _CACHED_NC
    if _CACHED_NC is None:
        _CACHED_NC = build_program()
    return _CACHED_NC


def _prep_in_maps(x, W_phi, b_phi, W1, b1, W2, b2, W3, b3):
    import ml_dtypes
    bf = ml_dtypes.bfloat16

    x = np.asarray(x, dtype=np.int64)
    Wd = np.asarray(W_phi, np.float64)
    bphid = np.asarray(b_phi, np.float64)
    W1d = np.asarray(W1, np.float64)
    W2d = np.asarray(W2, np.float64)
    W3d = np.asarray(W3, np.float64)
    b1d, b2d, b3d = (np.asarray(v, np.float64) for v in (b1, b2, b3))

    Wb = Wd.astype(np.float32).astype(bf).astype(np.float64)
    ubar = (N / C) * Wb.sum(0) + N * bphid
    off0v = (N * bphid - ubar).astype(np.float32)
    b1pv = (b1d + ubar @ W1d).astype(np.float32)
    c1v = np.maximum(b1pv, 0.0)
    b2pv = (b2d + c1v.astype(np.float64) @ W2d).astype(np.float32)
    c2v = np.maximum(b2pv, 0.0)
    b3pv = (b3d + c2v.astype(np.float64) @ W3d).astype(np.float32)

    # wselz[(lo,r) partition, (rv, h, d)] = Wb[32h+lo, d] iff r == rv
    wsel = Wb.astype(np.float32).reshape(HI, LO, PHI)     # [h, lo, d]
    wz = np.zeros((LO, 4, 4, HI, PHI), np.float32)        # [lo, r, rv, h, d]
    for r in range(4):
        wz[:, r, r, :, :] = wsel.transpose(1, 0, 2)
    wselzv = np.ascontiguousarray(
        wz.reshape(128, 4 * HI * PHI).astype(bf))

    w1v = np.ascontiguousarray(W1d.astype(np.float32).astype(bf))
    w2vv = np.ascontiguousarray(
        W2d.astype(np.float32).reshape(4, 128, 2, 128).transpose(1, 0, 2, 3)
        .reshape(128, 1024).astype(bf))
    w3v = np.ascontiguousarray(
        W3d.astype(np.float32).reshape(2, 128).T.astype(bf))

    cpk = np.zeros((128, 40), np.float32)
    cpk[:, 0:4] = np.minimum(b1pv, 0.0).reshape(4, 128).T
    cpk[:, 4:8] = (-c1v).reshape(4, 128).T
    cpk[:, 8:10] = np.minimum(b2pv, 0.0).reshape(2, 128).T
    cpk[:, 10:12] = (-c2v).reshape(2, 128).T
    cpk[:, 12] = off0v
    cpk[:, 13] = float(np.asarray(b3pv).reshape(-1)[0])
    cpk[:, 14] = 1.0
    cpk[:, 16:32] = -np.arange(HI, dtype=np.float32)
    cpk[:, 32:36] = b1pv.reshape(4, 128).T
    cpk[:, 36:38] = b2pv.reshape(2, 128).T
    shared = {
        "wselz": wselzv,
        "w1": w1v,
        "w2": w2vv,
        "w3": w3v,
        "cpack": np.ascontiguousarray(cpk),
    }

    lo_all = (x & 31).astype(np.float32)
    hi_all = (x >> 5).astype(np.float32)

    def quadpack(v):  # [512b, 512j] -> [128p, concat_blk(i, q_b, r)]
        a = v.T.reshape(NI, 128, 128, 4)             # [i, p, g, r]
        a = a.transpose(1, 0, 2, 3)                  # [p, i, g, r]
        parts = [np.ascontiguousarray(a[:, :, s:s + n, :]).reshape(128, -1)
                 for s, n in zip(QSTART, QS)]
        return np.ascontiguousarray(
            np.concatenate(parts, axis=1).astype(bf))

    maps = []
    for c in range(NCORES):
        sl = slice(c * BS, (c + 1) * BS)
        m = dict(shared)
        ql, qh = quadpack(lo_all[sl]), quadpack(hi_all[sl])
        parts = []
        for b in range(NBLK):
            o0, o1 = NI * QSTART[b] * 4, NI * (QSTART[b] + QS[b]) * 4
            parts.append(ql[:, o0:o1])
            parts.append(qh[:, o0:o1])
        m["xin"] = np.ascontiguousarray(np.concatenate(parts, axis=1))
        maps.append(m)
    return maps


def _unpermute(y_core):
    """Device col (blk | r, q-local) -> row b = (QSTART[blk]+q)*4 + r."""
    outp = np.empty(BS, np.float32)
    for blk in range(NBLK):
        QB = QS[blk]
        seg = y_core[4 * QSTART[blk]:4 * (QSTART[blk] + QB)].reshape(4, QB)
        for r in range(4):
            outp[(QSTART[blk] + np.arange(QB)) * 4 + r] = seg[r]
    return outp


def run(trace=False, tmpdir=None, **inputs):
    nc = _get_nc()
    in_maps = _prep_in_maps(**inputs)
    res = run_bass_kernel_spmd(nc, in_maps, core_ids=list(range(NCORES)),
                               trace=trace, tmpdir=tmpdir)
    y = np.concatenate([
        _unpermute(np.asarray(res.results[c]["out"]).reshape(BS))
        for c in range(NCORES)])
    return y.reshape(B, 1).astype(np.float32), res


def kernel(**inputs):
    y, _ = run(trace=False, **inputs)
    return y
